# revision 1
# baseline (speedup 1.0000x reference)
"""Sparse (conv-compressed) multi-head attention on 8 Trainium2 NeuronCores.

Entry point: kernel(**inputs) -> np.ndarray [4, 2048, 1024] float32.


Sharding: core c = 2*b + g  (b = batch 0..3, g = head-half 0..1).
Each core: batch b, heads [8g, 8g+8), all 2048 queries.
Final projection produces a partial (dv-half contraction); host sums pairs + bias.

Layout is fully transposed (channels on partitions):
  kT [1024, 2048] -> conv (as strided matmul, out-channel half, pair-AllGather)
  -> kcT [1024, 683] -> kpT [512, 683], vp_aug [683, 8*65] (ones col per head)
  qT [1024, 2048] -> qpT [512, 2048]
  per head h, q-tile m (512), j-chunk jc (128): sT = kpT_h.T @ qpT_h
  mask-add (host tiles) -> exp (ACT, scale=1/8) -> eT
  o_aug = vp_aug_h.T @ eT  (row 64 = softmax denom)
  o_n = o_aug[0:64] * (1/S broadcast via K=1 matmul)
  out_partial = o_nT.T @ WoT_half
"""
import sys
sys.path.insert(0, '/opt/trn_rl_repo')
import numpy as np
import concourse.bass as bass
import concourse.bacc as bacc
import concourse.mybir as mybir
from concourse import tile
from contextlib import ExitStack

f32 = mybir.dt.float32
f32r = mybir.dt.float32r
bf16 = mybir.dt.bfloat16
DT = bf16      # matmul operand dtype (all phases)
DT_ATT = bf16  # dtype for attention matmul operands (kpT/qpT/vp_aug/eT)
Exp = mybir.ActivationFunctionType.Exp

B, T, D, H = 4, 2048, 1024, 16
DH = 64
TC = 683          # compressed keys: 1 + 682
TCONV = 682
KK = 3 * D        # 3072 contraction for conv
DHALF = D // 2    # per-core head-half width
H8 = H // 2       # heads per core
NEG = -1.0e9
SCALE = DH ** -0.5  # 0.125

# conv N-splits (682 = 342 + 340, both >= 256 for f32r full rate)
CONV_NS = [(0, 342), (342, 340)]
# kp N-splits: fp32r matmul needs even N -> overlap col 341 (written twice)
KP_NS = [(0, 342), (341, 342)]

# attention q-tiles (4 x 512) and j-chunks (6 x 128, last = 43 rows)
NJ = 6
JROWS = [128, 128, 128, 128, 128, TC - 5 * 128]  # last = 43
JCS = {m: [jc for jc in range(NJ) if 384 * jc < 512 * (m + 1)] for m in range(4)}
# ragged tiles (m, jc) -> masked column count c1 (cols [0, c1) get mask add)
RAGGED = {}
for m in range(4):
    for jc in JCS[m]:
        pure = (384 * jc + 381 <= 512 * m)
        if not pure:
            RAGGED[(m, jc)] = min(512, 384 * jc + 381 - 512 * m)
C0 = {}
for m in range(4):
    for jc in JCS[m]:
        C0[(m, jc)] = max(0, 384 * jc - 512 * m)
RAGGED_LIST = sorted(RAGGED.keys())  # 8 tiles
assert len(RAGGED_LIST) == 8



def build_nc():
    nc = bacc.Bacc(None, target_bir_lowering=False, debug=False)

    qT = nc.dram_tensor("qT", [D, T], DT, kind="ExternalInput")
    kT = nc.dram_tensor("kT", [D, T], DT, kind="ExternalInput")
    WcH = nc.dram_tensor("WcH", [KK, DHALF], DT, kind="ExternalInput")
    WqT = nc.dram_tensor("WqT", [D, DHALF], DT, kind="ExternalInput")
    WkT = nc.dram_tensor("WkT", [D, DHALF], DT, kind="ExternalInput")
    WvT = nc.dram_tensor("WvT", [D, DHALF], DT, kind="ExternalInput")
    WoT = nc.dram_tensor("WoT", [DHALF, D], DT, kind="ExternalInput")
    MASK = nc.dram_tensor("MASK", [8, 128, 512], f32, kind="ExternalInput")
    K0H = nc.dram_tensor("K0H", [DHALF, 1], DT, kind="ExternalInput")
    SEL = nc.dram_tensor("SEL", [32, 2048], f32r, kind="ExternalInput")  # bcast selector
    OUT = nc.dram_tensor("out_p", [T, D], f32, kind="ExternalOutput")

    kc_half = nc.dram_tensor("kc_half", [DHALF, TC], DT)
    kc_full = nc.dram_tensor("kc_full", [D, TC], DT)

    with tile.TileContext(nc) as tc, ExitStack() as st:
        st.enter_context(nc.allow_low_precision("float32r is 4-byte fp32 storage"))
        pool = lambda **kw: st.enter_context(tc.tile_pool(**kw))
        p_big = pool(name="big", bufs=8)        # kT -> qpT + o_nT [128,2048]
        p_ws = pool(name="wstream", bufs=4)     # streamed weights [128,<=512]
        p_kc = pool(name="kc", bufs=8)          # kcT [128,683]
        p_kp = pool(name="kp", bufs=4)          # kpT [128,683]
        p_vpa = pool(name="vpa", bufs=6)        # vp_aug [128,520]
        p_mask = pool(name="mask", bufs=8)      # mask tiles [128,512] f32
        p_qt = pool(name="qt", bufs=12)         # qT slices [128,512]
        p_et = pool(name="et", bufs=8)          # eT [128,512]
        p_out = pool(name="outsb", bufs=3)      # out staging [128,1024] f32
        p_wo = pool(name="wo", bufs=8)          # WoT resident [128,512]
        p_small = pool(name="small", bufs=4)
        p_ont = pool(name="ont", bufs=16)       # o_nT split per (dv-chunk, m) [128,512]

        kt = [None] * 8

        def load_kt(ic):
            t = p_big.tile([128, T], DT, name="big", tag="big")
            nc.sync.dma_start(t[:], kT[128 * ic:128 * (ic + 1), :])
            kt[ic] = t

        # ---- conv: own out-channel half, K-contiguous, WcH streamed once ----
        with tc.tile_pool(name="ps_conv", bufs=8, space="PSUM") as ps_conv:
            ps_kc = {}
            for kk in range(24):
                if kk < 8:
                    load_kt(kk)
                wc_t = p_ws.tile([128, DHALF], DT, name="ws", tag="ws")
                nc.sync.dma_start(wc_t[:], WcH[128 * kk:128 * (kk + 1), :])
                r, ic = kk // 8, kk % 8
                for ni, (t0, tw) in enumerate(CONV_NS):
                    rhs = kt[ic][:, :3 * TCONV].rearrange(
                        "p (t r) -> p t r", r=3)[:, t0:t0 + tw, r]
                    for mc in range(4):
                        if kk == 0:
                            ps_kc[(ni, mc)] = ps_conv.tile(
                                [128, 342], f32, name="ps_kc", tag="ps_kc")
                        nc.tensor.matmul(
                            ps_kc[(ni, mc)][:, :tw],
                            wc_t[:, 128 * mc:128 * (mc + 1)],
                            rhs,
                            start=(kk == 0), stop=(kk == 23))
            for mc in range(4):
                t = p_kc.tile([128, TC], DT, name="kc", tag="kc")
                for ni, (t0, tw) in enumerate(CONV_NS):
                    nc.vector.tensor_copy(
                        t[:, 1 + t0:1 + t0 + tw], ps_kc[(ni, mc)][:, :tw])
                nc.sync.dma_start(kc_half[128 * mc:128 * (mc + 1), 1:], t[:, 1:])
        with nc.allow_non_contiguous_dma(reason="512x1 col write, 2KB total"):
            nc.sync.dma_start(kc_half[:, 0:1], K0H[:])

        nc.gpsimd.collective_compute(
            "AllGather", mybir.AluOpType.bypass,
            replica_groups=[[0, 1], [2, 3], [4, 5], [6, 7]],
            ins=[kc_half[:]], outs=[kc_full[:]],
        )

        kc = []
        for c in range(8):
            t = p_kc.tile([128, TC], DT, name="kc", tag="kc")
            nc.sync.dma_start(t[:], kc_full[128 * c:128 * (c + 1), :])
            kc.append(t)

        # ---- qp^T = WqT-lhsT @ qT  [512, 2048], 2 passes of 2 n-tiles ----
        qpt = [p_big.tile([128, T], DT_ATT, name="big", tag="big") for _ in range(4)]
        with tc.tile_pool(name="ps_qp", bufs=8, space="PSUM") as ps_pool:
            for npass in range(2):
                ps_qp = {}
                for kk in range(8):
                    wq_t = p_ws.tile([128, DHALF], DT, name="ws", tag="ws")
                    nc.sync.dma_start(wq_t[:], WqT[128 * kk:128 * (kk + 1), :])
                    for n in (2 * npass, 2 * npass + 1):
                        qt_t = p_qt.tile([128, 512], DT, name="qt", tag="qt")
                        nc.sync.dma_start(
                            qt_t[:], qT[128 * kk:128 * (kk + 1), 512 * n:512 * (n + 1)])
                        for m in range(4):
                            if kk == 0:
                                ps_qp[(m, n)] = ps_pool.tile(
                                    [128, 512], f32, name="ps_qp", tag="ps_qp")
                            nc.tensor.matmul(
                                ps_qp[(m, n)][:],
                                wq_t[:, 128 * m:128 * (m + 1)],
                                qt_t[:],
                                start=(kk == 0), stop=(kk == 7))
                for (m, n), ps in ps_qp.items():
                    nc.vector.tensor_copy(qpt[m][:, 512 * n:512 * (n + 1)], ps[:])

        # ---- vp_aug [683, 8*65]: vp = kcT-lhsT @ WvT, + ones columns ----
        ones_vpa = p_small.tile([128, 8], f32, name="ones_vpa", tag="ones_vpa", bufs=1)
        nc.vector.memset(ones_vpa[:], 1.0)
        vpa = [p_vpa.tile([128, H8 * 65], DT_ATT, name="vpa", tag="vpa")
               for _ in range(NJ)]
        with tc.tile_pool(name="ps_vp", bufs=6, space="PSUM") as ps_pool:
            ps_vp = {}
            for kk in range(8):
                wv_t = p_ws.tile([128, DHALF], DT, name="ws", tag="ws")
                nc.sync.dma_start(wv_t[:], WvT[128 * kk:128 * (kk + 1), :])
                for jb in range(NJ):
                    jr = JROWS[jb]
                    if kk == 0:
                        ps_vp[jb] = ps_pool.tile(
                            [128, 512], f32, name="ps_vp", tag="ps_vp")
                    nc.tensor.matmul(
                        ps_vp[jb][:jr, :],
                        kc[kk][:, 128 * jb:128 * jb + jr],
                        wv_t[:],
                        start=(kk == 0), stop=(kk == 7))
            for jb in range(NJ):
                jr = JROWS[jb]
                dst = vpa[jb][:jr, :].rearrange("p (h c) -> p h c", c=65)
                src = ps_vp[jb][:jr, :].rearrange("p (h c) -> p h c", c=64)
                nc.vector.tensor_copy(dst[:, :, 0:64], src[:])
                nc.vector.tensor_copy(
                    dst[:, :, 64:65],
                    ones_vpa[:jr, :].rearrange("p (h c) -> p h c", c=1))

        # ---- kp^T = WkT-lhsT @ kcT  [512, 683] ----
        kpt = [p_kp.tile([128, TC], DT_ATT, name="kp", tag="kp") for _ in range(4)]
        with tc.tile_pool(name="ps_kp", bufs=8, space="PSUM") as ps_pool:
            ps_kp = {}
            for kk in range(8):
                wk_t = p_ws.tile([128, DHALF], DT, name="ws", tag="ws")
                nc.sync.dma_start(wk_t[:], WkT[128 * kk:128 * (kk + 1), :])
                for m in range(4):
                    for ni, (t0, tw) in enumerate(KP_NS):
                        if kk == 0:
                            ps_kp[(m, ni)] = ps_pool.tile(
                                [128, 342], f32, name="ps_kp", tag="ps_kp")
                        nc.tensor.matmul(
                            ps_kp[(m, ni)][:, :tw],
                            wk_t[:, 128 * m:128 * (m + 1)],
                            kc[kk][:, t0:t0 + tw],
                            start=(kk == 0), stop=(kk == 7))
            for (m, ni), ps in ps_kp.items():
                t0, tw = KP_NS[ni]
                nc.vector.tensor_copy(kpt[m][:, t0:t0 + tw], ps[:, :tw])

        # ---- masks + constants ----
        mk = []
        for t_i in range(8):
            mt = p_mask.tile([128, 512], f32, name="mask", tag="mask")
            nc.sync.dma_start(mt[:], MASK[t_i])
            mk.append(mt)
        sel = p_small.tile([32, 2048], f32r, name="sel", tag="sel", bufs=1)
        nc.sync.dma_start(sel[:], SEL[:])

        # ---- attention (h outer, jc mid for lhsT reuse, m inner) ----
        # o_nt holds UNNORMALIZED head outputs; S_all collects denominators.
        o_nt = {(kk, m): p_ont.tile([128, 512], DT, name="ont", tag="ont")
                for kk in range(4) for m in range(4)}
        S_all = p_small.tile([32, 512], f32, name="s_all", tag="s_all", bufs=1)
        with tc.tile_pool(name="ps_att", bufs=3, space="PSUM") as ps_att:
            for h in range(H8):
                hc, off = h // 2, (h % 2) * 64
                po = {m: ps_att.tile([128, 512], f32, name="ps_o", tag="ps_o", bufs=4)
                      for m in range(4)}
                for jc in range(NJ):
                    jr = JROWS[jc]
                    for m in range(4):
                        if jc not in JCS[m]:
                            continue
                        key = (m, jc)
                        c0 = C0[key]
                        ps = ps_att.tile([128, 512], f32, name="ps_s", tag="ps_s", bufs=4)
                        nc.tensor.matmul(
                            ps[:jr, c0:],
                            kpt[hc][off:off + 64, 128 * jc:128 * jc + jr],
                            qpt[hc][off:off + 64, 512 * m + c0:512 * (m + 1)],
                            start=True, stop=True)
                        if key in RAGGED:
                            c1 = RAGGED[key]
                            ti = RAGGED_LIST.index(key)
                            nc.vector.tensor_add(
                                ps[:jr, c0:c1], ps[:jr, c0:c1], mk[ti][:jr, c0:c1])
                        et = p_et.tile([128, 512], DT_ATT, name="et", tag="et")
                        nc.scalar.activation(et[:jr, c0:], ps[:jr, c0:], Exp, scale=SCALE)
                        nc.tensor.matmul(
                            po[m][:65, c0:],
                            vpa[jc][:jr, 65 * h:65 * (h + 1)],
                            et[:jr, c0:],
                            start=(jc == 0), stop=(jc == JCS[m][-1]))
                for m in range(4):
                    idx = m * 8 + h
                    s_stage = p_small.tile([1, 512], f32, name="s_stage",
                                           tag="s_stage", bufs=4)
                    nc.vector.tensor_copy(s_stage[:], po[m][64:65, :])
                    nc.sync.dma_start(S_all[idx:idx + 1, :], s_stage[:])
                    nc.vector.tensor_copy(
                        o_nt[(hc, m)][off:off + 64, :], po[m][0:64, :])

        # ---- normalize + final, interleaved by m in ONE PSUM pool so the
        # final matmuls overlap the remaining broadcast/mult work ----
        R_all = p_small.tile([32, 512], f32r, name="r_all", tag="r_all", bufs=1)
        nc.vector.reciprocal(R_all[:], S_all[:])
        wot = {}
        for kk in range(4):
            for nn in range(2):
                wt = p_wo.tile([128, 512], DT, name="wo", tag="wo")
                nc.sync.dma_start(
                    wt[:], WoT[128 * kk:128 * (kk + 1), 512 * nn:512 * (nn + 1)])
                wot[(kk, nn)] = wt
        with tc.tile_pool(name="ps_nf", bufs=3, space="PSUM") as ps_nf:
            for m in range(4):
                for p in range(4 * m, 4 * m + 4):
                    bc = ps_nf.tile([128, 512], f32, name="ps_bc", tag="ps_bc", bufs=3)
                    nc.tensor.matmul(
                        bc[:], sel[:, 128 * p:128 * (p + 1)], R_all[:],
                        start=True, stop=True)
                    for half in range(2):
                        idx = 2 * p + half
                        mm, h = divmod(idx, 8)
                        hc, off = h // 2, (h % 2) * 64
                        dst = o_nt[(hc, mm)][off:off + 64, :]
                        nc.vector.tensor_mul(dst, dst, bc[64 * half:64 * half + 64, :])
                for mq in range(4 * m, 4 * m + 4):
                    ob = p_out.tile([128, D], f32, name="outsb", tag="outsb")
                    for nn in range(2):
                        pf = ps_nf.tile([128, 512], f32, name="ps_f", tag="ps_f", bufs=4)
                        for kk in range(4):
                            nc.tensor.matmul(
                                pf[:],
                                o_nt[(kk, mq // 4)][:, 128 * (mq % 4):128 * (mq % 4 + 1)],
                                wot[(kk, nn)][:],
                                start=(kk == 0), stop=(kk == 3))
                        nc.vector.tensor_copy(ob[:, 512 * nn:512 * (nn + 1)], pf[:])
                    nc.sync.dma_start(OUT[128 * mq:128 * (mq + 1), :], ob[:])

    return nc


def make_mask() -> np.ndarray:
    mask = np.zeros((8, 128, 512), dtype=np.float32)
    for t, (m, jc) in enumerate(RAGGED_LIST):
        q = 512 * m + np.arange(512)[None, :]
        j = 128 * jc + np.arange(128)[:, None]
        mask[t] = np.where(3 * j > q, NEG, 0.0).astype(np.float32)
    return mask


def make_sel() -> np.ndarray:
    m = np.arange(2048)
    k_of_m = 2 * (m // 128) + (m % 128) // 64
    sel = (np.arange(32)[:, None] == k_of_m[None, :]).astype(np.float32)
    return sel


def prep_inputs(q, k, Wq, Wk, Wv, Wo, conv_w):
    """Returns list of 8 in_maps (core c = 2b + g)."""
    import ml_dtypes
    bf = ml_dtypes.bfloat16
    Wc = np.ascontiguousarray(conv_w.transpose(2, 1, 0).reshape(KK, D))
    mask = make_mask()
    sel = make_sel()
    in_maps = []
    for c in range(8):
        b, g = c // 2, c % 2
        sl = slice(DHALF * g, DHALF * (g + 1))
        in_maps.append({
            "qT": np.ascontiguousarray(q[b].T).astype(bf),
            "kT": np.ascontiguousarray(k[b].T).astype(bf),
            "WcH": np.ascontiguousarray(Wc[:, sl]).astype(bf),
            "WqT": np.ascontiguousarray(Wq[sl, :].T).astype(bf),
            "WkT": np.ascontiguousarray(Wk[sl, :].T).astype(bf),
            "WvT": np.ascontiguousarray(Wv[sl, :].T).astype(bf),
            "WoT": np.ascontiguousarray(Wo[:, sl].T).astype(bf),
            "MASK": mask,
            "K0H": np.ascontiguousarray(k[b, 0, sl].reshape(DHALF, 1)).astype(bf),
            "SEL": sel,
        })
    return in_maps


def postprocess(results, bo):
    out = np.zeros((B, T, D), dtype=np.float32)
    for b in range(B):
        out[b] = (np.asarray(results[2 * b]["out_p"], dtype=np.float32)
                  + np.asarray(results[2 * b + 1]["out_p"], dtype=np.float32)
                  + bo[None, :])
    return out


_CACHED_NC = None


def kernel(q, k, v, Wq, Wk, Wv, Wo, bo, conv_w):
    """Full-input entry point. v is unused by the reference computation
    (V is replaced by the conv-compressed K)."""
    global _CACHED_NC
    from concourse.bass_utils import run_bass_kernel_spmd

    q = np.asarray(q, dtype=np.float32)
    k = np.asarray(k, dtype=np.float32)
    Wq = np.asarray(Wq, dtype=np.float32)
    Wk = np.asarray(Wk, dtype=np.float32)
    Wv = np.asarray(Wv, dtype=np.float32)
    Wo = np.asarray(Wo, dtype=np.float32)
    bo = np.asarray(bo, dtype=np.float32)
    conv_w = np.asarray(conv_w, dtype=np.float32)

    in_maps = prep_inputs(q, k, Wq, Wk, Wv, Wo, conv_w)
    if _CACHED_NC is None:
        nc = build_nc()
        nc.finalize()
        _CACHED_NC = nc
    res = run_bass_kernel_spmd(_CACHED_NC, in_maps, list(range(8)))
    return postprocess(res.results, bo)



# revision 6
# speedup vs baseline: 1.0694x; 1.0694x over previous
"""Sparse (conv-compressed) multi-head attention on 8 Trainium2 NeuronCores.

Entry point: kernel(**inputs) -> np.ndarray [4, 2048, 1024] float32.

Sharding: core c = 2*b + g  (b = batch 0..3, g = head-half 0..1).
Each core: batch b, heads [8g, 8g+8), all 2048 queries.
Final projection produces a partial (dv-half contraction); host sums pairs + bias.

v2 design (vs v1): the strided conv that builds the compressed keys kc is
FUSED into the Wk/Wv projections on the host:
    kc[j] = Wc3 @ k3(j-1)   (j>=1;  k3(t) = concat(k[3t], k[3t+1], k[3t+2]))
    kp[j] = Wk_hh @ kc[j] = (Wk_hh @ Wc3) @ k3(j-1) = Weff_k @ k3(j-1)
so each core computes kp/vp directly from its resident kT tiles with a
3072-deep contraction -- no conv intermediate, no cross-core AllGather.
Column j=0 (kc[0] = k[0]) is computed on the host and DMA'd in.
The causal mask is applied as a 0/1 bf16 MULTIPLY on eT after exp
(equivalent to the -1e9 additive mask, cheaper on the vector engine).
Attention runs q-tile-major (m outer) with per-m normalization + output
projection interleaved between the attention blocks so the PE stream
stays dense (TRN2 PE clock ramps 1.2 -> 2.4 GHz only under continuous load).
"""
import sys
sys.path.insert(0, '/opt/trn_rl_repo')
import numpy as np
import concourse.bass as bass
import concourse.bacc as bacc
import concourse.mybir as mybir
from concourse import tile
from contextlib import ExitStack

f32 = mybir.dt.float32
f32r = mybir.dt.float32r
bf16 = mybir.dt.bfloat16
DT = bf16
Exp = mybir.ActivationFunctionType.Exp

B, T, D, H = 4, 2048, 1024, 16
DH = 64
TC = 683          # compressed keys: 1 + 682
DHALF = D // 2    # per-core head-half width
H8 = H // 2       # heads per core
SCALE = DH ** -0.5  # 0.125

# kp n-splits over keys [1, 683)
KPN = [(1, 341), (342, 341)]

# attention q-tiles (4 x 512) and j-chunks (6 x 128, last = 43 rows)
NJ = 6
JROWS = [128, 128, 128, 128, 128, TC - 5 * 128]  # last = 43
JCS = {m: [jc for jc in range(NJ) if 384 * jc < 512 * (m + 1)] for m in range(4)}
C0 = {(m, jc): max(0, 384 * jc - 512 * m) for m in range(4) for jc in JCS[m]}
# ragged tiles (m, jc) -> first fully-visible column c1 (cols [c0, c1) get mask)
RAGGED = {}
for m in range(4):
    for jc in JCS[m]:
        if not (384 * jc + 381 <= 512 * m):
            RAGGED[(m, jc)] = min(512, 384 * jc + 381 - 512 * m)
RAGGED_LIST = sorted(RAGGED.keys())  # 8 tiles
assert len(RAGGED_LIST) == 8


def build_nc():
    nc = bacc.Bacc(None, target_bir_lowering=False, debug=False)

    qT = nc.dram_tensor("qT", [D, T], DT, kind="ExternalInput")
    kT = nc.dram_tensor("kT", [D, T], DT, kind="ExternalInput")
    WEK = nc.dram_tensor("WEK", [3 * D, DHALF], DT, kind="ExternalInput")
    WEV = nc.dram_tensor("WEV", [3 * D, DHALF], DT, kind="ExternalInput")
    WQT = nc.dram_tensor("WQT", [D, DHALF], DT, kind="ExternalInput")
    WOT = nc.dram_tensor("WOT", [DHALF, D], DT, kind="ExternalInput")
    MASKB = nc.dram_tensor("MASKB", [8, 128, 512], DT, kind="ExternalInput")
    SEL4 = nc.dram_tensor("SEL4", [4, 8, 128], f32r, kind="ExternalInput")
    KP0 = nc.dram_tensor("KP0", [DHALF, 1], DT, kind="ExternalInput")
    VP0R = nc.dram_tensor("VP0R", [1, 520], DT, kind="ExternalInput")
    OUT = nc.dram_tensor("out_p", [T, D], f32, kind="ExternalOutput")

    with tile.TileContext(nc) as tc, ExitStack() as st:
        st.enter_context(nc.allow_low_precision("bf16 matmuls, f32r recip bcast"))
        pool = lambda **kw: st.enter_context(tc.tile_pool(**kw))
        p_ktp = pool(name="ktp", bufs=8)      # padded kT tiles [128, 2052]
        p_w = pool(name="wstream", bufs=8)    # streamed weight chunks [128, 512]
        p_qt = pool(name="qt", bufs=6)        # qT chunks [128, 512]
        p_kpt = pool(name="kpt", bufs=4)      # kp^T [128, 683]
        p_vpa = pool(name="vpa", bufs=6)      # vp_aug [128, 520]
        p_qpt = pool(name="qpt", bufs=4)      # qp^T [128, 2048]
        p_mask = pool(name="mask", bufs=8)    # 0/1 bf16 masks [128, 512]
        p_et = pool(name="et", bufs=6)        # exp(scores) [128, 512]
        p_ont = pool(name="ont", bufs=16)     # unnormalized head outs [128, 512]
        p_wo = pool(name="wo", bufs=8)        # WoT resident [128, 512]
        p_out = pool(name="outsb", bufs=3)    # out staging [128, 1024] f32
        p_sm = pool(name="sm", bufs=4)        # per-m denominators [8, 512]
        p_ss = pool(name="ss", bufs=6)        # denom staging rows + recip
        p_small = pool(name="small", bufs=6)

        # ---- small loads on the scalar queue (sync queue is for the big streams)
        mk = []
        for ti in range(8):
            mt = p_mask.tile([128, 512], DT, name="mask", tag="mask")
            nc.scalar.dma_start(mt[:], MASKB[ti])
            mk.append(mt)
        sel = []
        for t4 in range(4):
            s_t = p_small.tile([8, 128], f32r, name="sel", tag="sel", bufs=4)
            nc.scalar.dma_start(s_t[:], SEL4[t4])
            sel.append(s_t)
        wot = {}
        for kk in range(4):
            for nn2 in range(2):
                wt = p_wo.tile([128, 512], DT, name="wo", tag="wo")
                nc.scalar.dma_start(
                    wt[:], WOT[128 * kk:128 * (kk + 1), 512 * nn2:512 * (nn2 + 1)])
                wot[(kk, nn2)] = wt
        vp0 = p_small.tile([1, 520], DT, name="vp0", tag="vp0", bufs=1)
        nc.scalar.dma_start(vp0[:], VP0R[:])

        # dedicated result tiles
        kpt = [p_kpt.tile([128, TC], DT, name="kpt", tag="kpt") for _ in range(4)]
        with nc.allow_non_contiguous_dma(reason="kp col-0 writes, 1KB total"):
            for m in range(4):
                nc.scalar.dma_start(kpt[m][:, 0:1], KP0[128 * m:128 * (m + 1), :])
        vpa = [p_vpa.tile([128, 520], DT, name="vpa", tag="vpa") for _ in range(NJ)]
        qpt = [p_qpt.tile([128, T], DT, name="qpt", tag="qpt") for _ in range(4)]
        o_nt = {(kk, m): p_ont.tile([128, 512], DT, name="ont", tag="ont")
                for kk in range(4) for m in range(4)}
        ones_vpa = p_small.tile([128, 8], f32, name="ones_vpa", tag="ones_vpa",
                                bufs=1)
        nc.vector.memset(ones_vpa[:], 1.0)

        # ---- kp^T fused: kpt[m][:, j] = (Wk_hh @ Wc3) @ k3(j-1), j in [1, 683)
        ktp = []
        with tc.tile_pool(name="ps_kp", bufs=8, space="PSUM") as ps_pool:
            ps_kp = {}
            for ic in range(8):
                t = p_ktp.tile([128, 2052], DT, name="ktp", tag="ktp")
                nc.vector.memset(t[:, 0:3], 0.0)
                nc.sync.dma_start(t[:, 3:2051], kT[128 * ic:128 * (ic + 1), :])
                ktp.append(t)
                for r in range(3):
                    wt = p_w.tile([128, DHALF], DT, name="ws", tag="ws")
                    nc.sync.dma_start(
                        wt[:], WEK[1024 * r + 128 * ic:1024 * r + 128 * (ic + 1), :])
                    vv = t[:, r:r + 3 * TC].rearrange("p (t s) -> p t s", s=3)
                    first = (ic == 0 and r == 0)
                    last = (ic == 7 and r == 2)
                    for m in range(4):
                        for ni, (t0, tw) in enumerate(KPN):
                            if first:
                                ps_kp[(m, ni)] = ps_pool.tile(
                                    [128, 341], f32, name="ps_kp", tag="ps_kp")
                            nc.tensor.matmul(
                                ps_kp[(m, ni)][:, :tw],
                                wt[:, 128 * m:128 * (m + 1)],
                                vv[:, t0:t0 + tw, 0],
                                start=first, stop=last)
            for (m, ni), ps in ps_kp.items():
                t0, tw = KPN[ni]
                nc.scalar.copy(kpt[m][:, t0:t0 + tw], ps[:, :tw])

        # ---- vp_aug fused: vpa[jb][j, 65h+c] = vp[128jb+j, 64h+c], col 65h+64 = 1
        with tc.tile_pool(name="ps_vp", bufs=6, space="PSUM") as ps_pool:
            ps_vp = {}
            for ic in range(8):
                for r in range(3):
                    wt = p_w.tile([128, DHALF], DT, name="ws", tag="ws")
                    nc.sync.dma_start(
                        wt[:], WEV[1024 * r + 128 * ic:1024 * r + 128 * (ic + 1), :])
                    vv = ktp[ic][:, r:r + 3 * TC].rearrange("p (t s) -> p t s", s=3)
                    first = (ic == 0 and r == 0)
                    last = (ic == 7 and r == 2)
                    for jb in range(NJ):
                        jr = JROWS[jb]
                        if first:
                            ps_vp[jb] = ps_pool.tile(
                                [128, 512], f32, name="ps_vp", tag="ps_vp")
                        nc.tensor.matmul(
                            ps_vp[jb][:jr, :],
                            vv[:, 128 * jb:128 * jb + jr, 0],
                            wt[:],
                            start=first, stop=last)
            for jb in range(NJ):
                jr = JROWS[jb]
                dst = vpa[jb][:jr].rearrange("p (h c) -> p h c", c=65)
                src = ps_vp[jb][:jr].rearrange("p (h c) -> p h c", c=64)
                nc.vector.tensor_copy(dst[:, :, 0:64], src[:])
                nc.vector.tensor_copy(
                    dst[:, :, 64:65],
                    ones_vpa[:jr, :].rearrange("p (h c) -> p h c", c=1))
            # row 0 (key 0) comes from the host: kc[0] = k[0]
            nc.vector.tensor_copy(vpa[0][0:1, :], vp0[:])

        # ---- qp^T = WqT-lhsT @ qT  [512, 2048], 2 passes of 2 n-tiles ----
        with tc.tile_pool(name="ps_qp", bufs=8, space="PSUM") as ps_pool:
            for npass in range(2):
                ps_qp = {}
                for kk in range(8):
                    wq_t = p_w.tile([128, DHALF], DT, name="ws", tag="ws")
                    nc.sync.dma_start(wq_t[:], WQT[128 * kk:128 * (kk + 1), :])
                    for n in (2 * npass, 2 * npass + 1):
                        qt_t = p_qt.tile([128, 512], DT, name="qt", tag="qt")
                        nc.sync.dma_start(
                            qt_t[:],
                            qT[128 * kk:128 * (kk + 1), 512 * n:512 * (n + 1)])
                        for m in range(4):
                            if kk == 0:
                                ps_qp[(m, n)] = ps_pool.tile(
                                    [128, 512], f32, name="ps_qp", tag="ps_qp")
                            nc.tensor.matmul(
                                ps_qp[(m, n)][:],
                                wq_t[:, 128 * m:128 * (m + 1)],
                                qt_t[:],
                                start=(kk == 0), stop=(kk == 7))
                for (m, n), ps in ps_qp.items():
                    if m % 2 == 0:
                        nc.scalar.copy(qpt[m][:, 512 * n:512 * (n + 1)], ps[:])
                    else:
                        nc.vector.tensor_copy(qpt[m][:, 512 * n:512 * (n + 1)], ps[:])

        # ---- attention (m outer) with interleaved per-m finalize ----
        S_m = [p_sm.tile([8, 512], f32, name="sm", tag="sm") for _ in range(4)]

        def att(m, ps_att):
            js = JCS[m]
            for h in range(H8):
                hc, off = h // 2, (h % 2) * 64
                po = ps_att.tile([128, 512], f32, name="ps_o", tag="ps_o", bufs=2)
                for jc in js:
                    jr, c0 = JROWS[jc], C0[(m, jc)]
                    ps = ps_att.tile([128, 512], f32, name="ps_s", tag="ps_s",
                                     bufs=2)
                    nc.tensor.matmul(
                        ps[:jr, c0:],
                        kpt[hc][off:off + 64, 128 * jc:128 * jc + jr],
                        qpt[hc][off:off + 64, 512 * m + c0:512 * (m + 1)],
                        start=True, stop=True)
                    et = p_et.tile([128, 512], DT, name="et", tag="et")
                    nc.scalar.activation(et[:jr, c0:], ps[:jr, c0:], Exp,
                                         scale=SCALE)
                    if (m, jc) in RAGGED:
                        c1 = RAGGED[(m, jc)]
                        ti = RAGGED_LIST.index((m, jc))
                        nc.gpsimd.tensor_mul(
                            et[:jr, c0:c1], et[:jr, c0:c1], mk[ti][:jr, c0:c1])
                    nc.tensor.matmul(
                        po[:65, c0:],
                        vpa[jc][:jr, 65 * h:65 * (h + 1)],
                        et[:jr, c0:],
                        start=(jc == js[0]), stop=(jc == js[-1]))
                nc.vector.tensor_copy(o_nt[(hc, m)][off:off + 64, :], po[0:64, :])
                ss = p_ss.tile([1, 512], f32, name="ss", tag="ss", bufs=4)
                nc.vector.tensor_copy(ss[:], po[64:65, :])
                nc.sync.dma_start(S_m[m][h:h + 1, :], ss[:])

        def fin(m, ps_att):
            rm = p_ss.tile([8, 512], f32r, name="rm", tag="rm", bufs=2)
            nc.vector.reciprocal(rm[:], S_m[m][:])
            for t4 in range(4):
                bc = ps_att.tile([128, 512], f32, name="ps_bc", tag="ps_bc",
                                 bufs=2)
                nc.tensor.matmul(bc[:], sel[t4][:], rm[:], start=True, stop=True)
                for half in range(2):
                    dst = o_nt[(t4, m)][64 * half:64 * half + 64, :]
                    nc.vector.tensor_mul(
                        dst, dst, bc[64 * half:64 * half + 64, :])
            for mq in range(4):
                ob = p_out.tile([128, D], f32, name="outsb", tag="outsb")
                for nn2 in range(2):
                    pf = ps_att.tile([128, 512], f32, name="ps_f", tag="ps_f",
                                     bufs=2)
                    for kk in range(4):
                        nc.tensor.matmul(
                            pf[:],
                            o_nt[(kk, m)][:, 128 * mq:128 * (mq + 1)],
                            wot[(kk, nn2)][:],
                            start=(kk == 0), stop=(kk == 3))
                    nc.vector.tensor_copy(ob[:, 512 * nn2:512 * (nn2 + 1)], pf[:])
                nc.gpsimd.dma_start(
                    OUT[512 * m + 128 * mq:512 * m + 128 * (mq + 1), :], ob[:])

        with tc.tile_pool(name="ps_att", bufs=2, space="PSUM") as ps_att:
            att(0, ps_att)
            att(1, ps_att)
            fin(0, ps_att)
            att(2, ps_att)
            fin(1, ps_att)
            att(3, ps_att)
            fin(2, ps_att)
            fin(3, ps_att)

    return nc


def make_maskb():
    import ml_dtypes
    mask = np.zeros((8, 128, 512), dtype=np.float32)
    for t, (m, jc) in enumerate(RAGGED_LIST):
        qq = 512 * m + np.arange(512)[None, :]
        jj = 128 * jc + np.arange(128)[:, None]
        mask[t] = (3 * jj <= qq).astype(np.float32)  # 1.0 where visible
    return mask.astype(ml_dtypes.bfloat16)


def make_sel4():
    sel = np.zeros((4, 8, 128), dtype=np.float32)
    for t in range(4):
        for r in range(128):
            sel[t, 2 * t + r // 64, r] = 1.0
    return sel


def prep_inputs(q, k, Wq, Wk, Wv, Wo, conv_w):
    """Returns list of 8 in_maps (core c = 2b + g)."""
    import ml_dtypes
    bf = ml_dtypes.bfloat16
    # Wc3T[r*1024 + i, o] = conv_w[o, i, r]  (so kc[j] = Wc3T.T @ k3(j-1))
    Wc3T = np.ascontiguousarray(
        conv_w.transpose(2, 1, 0).reshape(3 * D, D)).astype(np.float32)
    maskb = make_maskb()
    sel4 = make_sel4()
    halves = []
    for g in range(2):
        sl = slice(DHALF * g, DHALF * (g + 1))
        WEKg = np.ascontiguousarray((Wc3T @ Wk[sl].T)).astype(bf)
        WEVg = np.ascontiguousarray((Wc3T @ Wv[sl].T)).astype(bf)
        halves.append((sl, WEKg, WEVg))
    in_maps = []
    for c in range(8):
        b, g = c // 2, c % 2
        sl, WEKg, WEVg = halves[g]
        kp0 = (Wk[sl] @ k[b, 0]).astype(np.float32).reshape(DHALF, 1)
        vp0 = (Wv[sl] @ k[b, 0]).astype(np.float32)
        vp0r = np.zeros((8, 65), np.float32)
        vp0r[:, :64] = vp0.reshape(8, 64)
        vp0r[:, 64] = 1.0
        in_maps.append({
            "qT": np.ascontiguousarray(q[b].T).astype(bf),
            "kT": np.ascontiguousarray(k[b].T).astype(bf),
            "WEK": WEKg,
            "WEV": WEVg,
            "WQT": np.ascontiguousarray(Wq[sl, :].T).astype(bf),
            "WOT": np.ascontiguousarray(Wo[:, sl].T).astype(bf),
            "MASKB": maskb,
            "SEL4": sel4,
            "KP0": kp0.astype(bf),
            "VP0R": vp0r.reshape(1, 520).astype(bf),
        })
    return in_maps


def postprocess(results, bo):
    out = np.zeros((B, T, D), dtype=np.float32)
    for b in range(B):
        out[b] = (np.asarray(results[2 * b]["out_p"], dtype=np.float32)
                  + np.asarray(results[2 * b + 1]["out_p"], dtype=np.float32)
                  + bo[None, :])
    return out


_CACHED_NC = None


def kernel(q, k, v, Wq, Wk, Wv, Wo, bo, conv_w):
    """Full-input entry point. v is unused by the reference computation
    (V is replaced by the conv-compressed K)."""
    global _CACHED_NC
    from concourse.bass_utils import run_bass_kernel_spmd

    q = np.asarray(q, dtype=np.float32)
    k = np.asarray(k, dtype=np.float32)
    Wq = np.asarray(Wq, dtype=np.float32)
    Wk = np.asarray(Wk, dtype=np.float32)
    Wv = np.asarray(Wv, dtype=np.float32)
    Wo = np.asarray(Wo, dtype=np.float32)
    bo = np.asarray(bo, dtype=np.float32)
    conv_w = np.asarray(conv_w, dtype=np.float32)

    in_maps = prep_inputs(q, k, Wq, Wk, Wv, Wo, conv_w)
    if _CACHED_NC is None:
        nc = build_nc()
        nc.finalize()
        _CACHED_NC = nc
    res = run_bass_kernel_spmd(_CACHED_NC, in_maps, list(range(8)))
    return postprocess(res.results, bo)
